# revision 13
# baseline (speedup 1.0000x reference)
"""Trainium2 Bass kernel for nn_DenseNetCmaxGatedB2 (gated pooling block).

Computation (per batch, per channel, depthwise):
  out = maxpool3x3s2(x) * (dwconv_s2(x, maxgate) + mb)
  g0  = sigmoid(dwconv_s2(x, pgates[...,0]) + gbs[:,0])
  n0  = g0*p0 + (1-g0)*p1           p_k = dwconv_s2(x, pconvs[...,k]) + pbs[:,k]
  g1  = sigmoid(dwconv_s2(x, pgates[...,2]) + gbs[:,1])
  n1  = g1*p2 + (1-g1)*p3
  g   = sigmoid(dwconv_s1(n0, pgates[...,2]) + gbs[:,2])
  out = out + n0*g + n1*(1-g)

Sharding: pure data parallel over batch (16 -> 2 per core, 8 cores).

Layout: channels on SBUF partitions (128 per plane; 2 batches x 2
channel-blocks = 4 planes per core).  Each conv tap is a fused
scalar_tensor_tensor MAC (acc = x_shift * w[c] + acc) with the per-channel
weight as the per-partition fp32 scalar operand.  For bf16 compute, x is
deinterleaved once per plane into even/odd row x col parity planes so
every tap reads with unit stride.  scalar_tensor_tensor / tensor_tensor
are not legal Pool-engine opcodes on TRN2, so all elementwise compute
runs on VectorE; ScalarE does the deinterleave, the first tap of each
conv (fused scale+bias) and the sigmoids.
"""

import contextlib
import sys

sys.path.insert(0, "/opt/trn_rl_repo")

import numpy as np

import concourse.bass as bass  # noqa: E402,F401
import concourse.mybir as mybir  # noqa: E402
from concourse import bacc  # noqa: E402
from concourse.tile import TileContext  # noqa: E402
from concourse.bass_utils import run_bass_kernel_spmd  # noqa: E402

N_CORES = 8
B, C, H = 16, 256, 128
HO = H // 2
BS = B // N_CORES  # batches per core
F32 = mybir.dt.float32
BF16 = mybir.dt.bfloat16
AF = mybir.ActivationFunctionType
OP = mybir.AluOpType

# (di, dj) tap order; (1,1) handled by ScalarE with fused scale+bias.
TAPS8 = [(0, 0), (0, 1), (0, 2), (1, 0), (1, 2), (2, 0), (2, 1), (2, 2)]


def _build(dt, reps=1):
    """Build the SPMD program for one core (2 batches, full channels).

    reps>1 wraps the per-plane pipeline in a hardware loop recomputing
    the same outputs; used only for wall-clock timing."""
    nc = bacc.Bacc("TRN2", target_bir_lowering=False, debug=False, num_devices=N_CORES)

    x_d = nc.dram_tensor("x", [BS, C, H * H], F32, kind="ExternalInput")
    mg_d = nc.dram_tensor("maxgate", [C, 9], F32, kind="ExternalInput")
    mb_d = nc.dram_tensor("mb", [C, 1], F32, kind="ExternalInput")
    pc_d = nc.dram_tensor("pconvs", [C, 36], F32, kind="ExternalInput")
    pb_d = nc.dram_tensor("pbs", [C, 4], F32, kind="ExternalInput")
    pg_d = nc.dram_tensor("pgates", [C, 27], F32, kind="ExternalInput")
    gb_d = nc.dram_tensor("gbs", [C, 3], F32, kind="ExternalInput")
    out_d = nc.dram_tensor("out", [BS, C, HO * HO], F32, kind="ExternalOutput")

    bf = dt == BF16
    V = nc.vector

    with TileContext(nc) as tc:
        with contextlib.ExitStack() as ctx:
            wp = ctx.enter_context(tc.tile_pool(name="w", bufs=1))
            xp = ctx.enter_context(tc.tile_pool(name="xp", bufs=1))
            pp = ctx.enter_context(tc.tile_pool(name="pp", bufs=2))
            ppz = ctx.enter_context(tc.tile_pool(name="ppz", bufs=1))
            ap = ctx.enter_context(tc.tile_pool(name="ap", bufs=1))
            op_ = ctx.enter_context(tc.tile_pool(name="op", bufs=2))

            # ---- weights / biases (fp32 per-partition scalars), per cblock
            W = []
            for cb in range(2):
                sl = slice(cb * 128, (cb + 1) * 128)
                wmg = wp.tile([128, 9], F32, tag=f"wmg{cb}")
                wpc = wp.tile([128, 36], F32, tag=f"wpc{cb}")
                wpg = wp.tile([128, 27], F32, tag=f"wpg{cb}")
                bmb = wp.tile([128, 1], F32, tag=f"bmb{cb}")
                bpb = wp.tile([128, 4], F32, tag=f"bpb{cb}")
                bgb = wp.tile([128, 3], F32, tag=f"bgb{cb}")
                nc.sync.dma_start(wmg[:], mg_d[sl, :])
                nc.sync.dma_start(wpc[:], pc_d[sl, :])
                nc.sync.dma_start(wpg[:], pg_d[sl, :])
                nc.sync.dma_start(bmb[:], mb_d[sl, :])
                nc.sync.dma_start(bpb[:], pb_d[sl, :])
                nc.sync.dma_start(bgb[:], gb_d[sl, :])

                def s(t, i):
                    return t[:, i : i + 1]

                def mk(wt, stride_, k):
                    return lambda di, dj, wt=wt, stride_=stride_, k=k: s(
                        wt, (di * 3 + dj) * stride_ + k
                    )

                W.append(
                    dict(
                        cm=(mk(wmg, 1, 0), s(bmb, 0)),
                        g0=(mk(wpg, 3, 0), s(bgb, 0)),
                        p0=(mk(wpc, 4, 0), s(bpb, 0)),
                        p1=(mk(wpc, 4, 1), s(bpb, 1)),
                        g1=(mk(wpg, 3, 2), s(bgb, 1)),
                        p2=(mk(wpc, 4, 2), s(bpb, 2)),
                        p3=(mk(wpc, 4, 3), s(bpb, 3)),
                        nd=(mk(wpg, 3, 2), s(bgb, 2)),
                    )
                )

            tmp_pool = ctx.enter_context(tc.tile_pool(name="tmp", bufs=2))

            def conv_s2(acc3, planes, wfn, bias):
                """Stride-2 3x3 depthwise conv into acc3 [128,64,64].

                scalar_tensor_tensor only has a 1x DVE uop, so instead each
                tap is a tensor_scalar product (4x mode in bf16) plus a
                tensor_tensor accumulate (2x mode) — ~35% fewer DVE cycles
                than the 1x fused MAC."""
                nc.scalar.activation(
                    acc3, planes["ee"][:], AF.Identity, bias=bias, scale=wfn(1, 1)
                )
                for di, dj in TAPS8:
                    rsel = "e" if di == 1 else "o"
                    csel = {0: "z", 1: "e", 2: "o"}[dj]
                    p = planes[rsel + csel]
                    i0 = 1 if di == 0 else 0
                    pin = p[:, 0 : 64 - i0, 0:64]
                    po = acc3[:, i0:64, :]
                    t = tmp_pool.tile([128, 64, 64], dt, tag="t", bufs=3, name="t")
                    tv = t[:, 0 : 64 - i0, :]
                    V.tensor_scalar(tv, pin, wfn(di, dj), None, OP.mult)
                    V.tensor_tensor(po, po, tv, OP.add)

            def conv_s2_strided(acc3, xv, wfn, bias):
                """fp32 path: taps read x [128,128,128] directly (strided)."""
                nc.scalar.activation(
                    acc3, xv[:, 0:128:2, 0:128:2], AF.Identity, bias=bias,
                    scale=wfn(1, 1),
                )
                for di, dj in TAPS8:
                    i0 = 1 if di == 0 else 0
                    j0 = 1 if dj == 0 else 0
                    r0 = di - 1 + 2 * i0
                    c0 = dj - 1 + 2 * j0
                    pin = xv[:, r0:128:2, c0:128:2][:, 0 : 64 - i0, 0 : 64 - j0]
                    po = acc3[:, i0:64, j0:64]
                    V.scalar_tensor_tensor(po, pin, wfn(di, dj), po, OP.mult, OP.add)

            def plane(b, cb):
                sl = slice(cb * 128, (cb + 1) * 128)
                w = W[cb]

                X = xp.tile([128, H * H], dt, tag="X", name="X")
                if bf:
                    nc.gpsimd.dma_start(X[:], x_d[b, sl, :])  # casts f32->bf16
                else:
                    nc.sync.dma_start(X[:], x_d[b, sl, :])
                xv = X[:].rearrange("p (r c) -> p r c", r=H)

                planes = None
                if bf:
                    pee = pp.tile([128, 64, 64], dt, tag="pee", name="pee")
                    peo = pp.tile([128, 64, 64], dt, tag="peo", name="peo")
                    poe = pp.tile([128, 64, 64], dt, tag="poe", name="poe")
                    poo = pp.tile([128, 64, 64], dt, tag="poo", name="poo")
                    pez = ppz.tile([128, 64, 65], dt, tag="pez", name="pez")
                    poz = ppz.tile([128, 64, 65], dt, tag="poz", name="poz")
                    nc.scalar.copy(pee[:], xv[:, 0:128:2, 0:128:2])
                    nc.scalar.copy(peo[:], xv[:, 0:128:2, 1:128:2])
                    nc.scalar.copy(poe[:], xv[:, 1:128:2, 0:128:2])
                    nc.scalar.copy(poo[:], xv[:, 1:128:2, 1:128:2])
                    nc.gpsimd.memset(pez[:, :, 0:1], 0)
                    nc.gpsimd.memset(poz[:, :, 0:1], 0)
                    nc.scalar.copy(pez[:, :, 1:65], xv[:, 0:128:2, 1:128:2])
                    nc.scalar.copy(poz[:, :, 1:65], xv[:, 1:128:2, 1:128:2])
                    planes = dict(ee=pee, eo=peo, oe=poe, oo=poo, ez=pez, oz=poz)

                def conv(acc3, key):
                    wfn, bias = w[key]
                    if bf:
                        conv_s2(acc3, planes, wfn, bias)
                    else:
                        conv_s2_strided(acc3, xv, wfn, bias)

                cm = ap.tile([128, 64, 64], dt, tag="A", name="cm")
                conv(cm[:], "cm")

                # maxpool via tensor_tensor max chain
                mp = ap.tile([128, 64, 64], dt, tag="B", name="mp")
                m3 = mp[:]
                rest = [(0, 0), (0, 1), (0, 2), (1, 0), (2, 0), (2, 1), (2, 2)]
                if bf:
                    V.tensor_tensor(m3, planes["ee"][:], planes["eo"][:], OP.max)
                    for di, dj in rest:
                        rsel = "e" if di == 1 else "o"
                        csel = {0: "o", 1: "e", 2: "o"}[dj]
                        p = planes[rsel + csel]
                        i0 = 1 if di == 0 else 0
                        j0 = 1 if dj == 0 else 0
                        pin = p[:, 0 : 64 - i0, 0 : 64 - j0]
                        po = m3[:, i0:64, j0:64]
                        V.tensor_tensor(po, po, pin, OP.max)
                else:
                    V.tensor_tensor(
                        m3, xv[:, 0:128:2, 0:128:2], xv[:, 0:128:2, 1:128:2], OP.max
                    )
                    for di, dj in rest:
                        i0 = 1 if di == 0 else 0
                        j0 = 1 if dj == 0 else 0
                        r0 = di - 1 + 2 * i0
                        c0 = dj - 1 + 2 * j0
                        pin = xv[:, r0:128:2, c0:128:2][:, 0 : 64 - i0, 0 : 64 - j0]
                        po = m3[:, i0:64, j0:64]
                        V.tensor_tensor(po, po, pin, OP.max)

                # mpcm = maxpool * cm   (keep in B)
                V.tensor_tensor(m3, m3, cm[:], OP.mult)

                g0 = ap.tile([128, 64, 64], dt, tag="A2", name="g0")
                conv(g0[:], "g0")
                nc.scalar.activation(g0[:], g0[:], AF.Sigmoid)

                p0 = ap.tile([128, 64, 64], dt, tag="C", name="p0")
                conv(p0[:], "p0")
                p1 = ap.tile([128, 64, 64], dt, tag="D", name="p1")
                conv(p1[:], "p1")

                # n0 = p1 + g0*(p0-p1), stored zero-padded [64,66]
                n0z = ap.tile([128, 64, 66], dt, tag="E", name="n0z")
                V.tensor_tensor(p0[:], p0[:], p1[:], OP.subtract)
                V.tensor_tensor(p0[:], p0[:], g0[:], OP.mult)
                nc.gpsimd.memset(n0z[:, :, 0:1], 0)
                nc.gpsimd.memset(n0z[:, :, 65:66], 0)
                n0 = n0z[:, :, 1:65]
                V.tensor_tensor(n0, p0[:], p1[:], OP.add)

                g1 = ap.tile([128, 64, 64], dt, tag="A2", name="g1")
                conv(g1[:], "g1")
                nc.scalar.activation(g1[:], g1[:], AF.Sigmoid)
                p2 = ap.tile([128, 64, 64], dt, tag="C", name="p2")
                conv(p2[:], "p2")
                p3 = ap.tile([128, 64, 64], dt, tag="D", name="p3")
                conv(p3[:], "p3")

                V.tensor_tensor(p2[:], p2[:], p3[:], OP.subtract)
                V.tensor_tensor(p2[:], p2[:], g1[:], OP.mult)
                V.tensor_tensor(p2[:], p2[:], p3[:], OP.add)
                n1 = p2  # tag C

                # node-stage gate: stride-1 conv over padded n0
                gc = ap.tile([128, 64, 64], dt, tag="A2", name="gc")
                wfn, bias = w["nd"]
                nc.scalar.activation(
                    gc[:], n0z[:, 0:64, 1:65], AF.Identity, bias=bias, scale=wfn(1, 1)
                )
                for di, dj in TAPS8:
                    i0 = 1 if di == 0 else 0
                    i1 = 1 if di == 2 else 0
                    r0n = di - 1 + i0
                    pin = n0z[:, r0n : r0n + 64 - i0 - i1, dj : dj + 64]
                    po = gc[:, i0 : 64 - i1, :]
                    V.scalar_tensor_tensor(po, pin, wfn(di, dj), po, OP.mult, OP.add)
                nc.scalar.activation(gc[:], gc[:], AF.Sigmoid)

                # out = mpcm + n1 + g*(n0-n1)
                o = op_.tile([128, 64, 64], dt, tag="O", name="o")
                V.tensor_tensor(o[:], n0, n1[:], OP.subtract)
                V.tensor_tensor(o[:], o[:], gc[:], OP.mult)
                V.tensor_tensor(o[:], o[:], n1[:], OP.add)
                V.tensor_tensor(o[:], o[:], m3, OP.add)

                oflat = o[:].rearrange("p a b -> p (a b)")
                if bf:
                    nc.gpsimd.dma_start(out_d[b, sl, :], oflat)  # cast back
                else:
                    nc.sync.dma_start(out_d[b, sl, :], oflat)

            rep_ctx = tc.For_i(0, reps, 1) if reps > 1 else contextlib.nullcontext()
            with rep_ctx:
                for b in range(BS):
                    for cb in range(2):
                        plane(b, cb)

    nc.compile()
    return nc


_NC_CACHE = {}


def _get_nc(dt, reps=1):
    key = (str(dt), reps)
    if key not in _NC_CACHE:
        _NC_CACHE[key] = _build(dt, reps)
    return _NC_CACHE[key]


def _in_maps(x, maxgate, mb, pconvs, pbs, pgates, gbs):
    x = np.ascontiguousarray(np.asarray(x, np.float32))
    maps = []
    for i in range(N_CORES):
        maps.append(
            dict(
                x=x[i * BS : (i + 1) * BS].reshape(BS, C, H * H),
                maxgate=np.asarray(maxgate, np.float32).reshape(C, 9),
                mb=np.asarray(mb, np.float32).reshape(C, 1),
                pconvs=np.asarray(pconvs, np.float32).reshape(C, 36),
                pbs=np.asarray(pbs, np.float32).reshape(C, 4),
                pgates=np.asarray(pgates, np.float32).reshape(C, 27),
                gbs=np.asarray(gbs, np.float32).reshape(C, 3),
            )
        )
    return maps


def kernel(x, maxgate, mb, pconvs, pbs, pgates, gbs):
    nc = _get_nc(BF16)
    maps = _in_maps(x, maxgate, mb, pconvs, pbs, pgates, gbs)
    res = run_bass_kernel_spmd(nc, maps, list(range(N_CORES)))
    return np.concatenate(
        [r["out"].reshape(BS, C, HO, HO) for r in res.results], axis=0
    )
